# revision 1
# baseline (speedup 1.0000x reference)
"""Trainium2 Bass kernel for CombinedRankingLoss (BCE + pairwise margin ranking).

Full inputs: logits/labels/weights [64, 1024, 1] f32. Output: scalar f32.

Data-parallel over batch: 8 cores x 8 batches. Pairwise term per batch
    T_b = sum_{i in pos} sum_{j in neg} relu((v_j + M) - v_i)
computed via a SORTED-BAND decomposition (the loss is invariant to per-batch
candidate order, and sorting is host-side layout prep):
  - host sorts pos ascending (a) and neg+M ascending (b) per batch;
  - pos ranks go to partitions in CH=5 chunks of 128 (rank 128c+p -> partition
    p, segment c of the tile);
  - for chunk c only the neg ranks in [W0_c, W0_c+need_c) can pair
    NON-trivially with the chunk (W0_c/hi_c from searchsorted): below-window
    j have b_j <= min_a (relu = 0), above-window j have b_j >= max_a (relu
    linear -> closed form);
  - ONE PE matmul per region builds psum[p, S_c+f] = s*(b[W0_c+f] - a[128c+p])
    directly: b rows (bf16 hi+lo, selector 1) add the b values, per-chunk
    "a rows" (selector -s*a hi+lo, b2 carrying the segment indicator)
    subtract a. s = 1/n_pairs(batch) folded into all values so batches can
    share accumulator columns;
  - the linear above-window remainder sum_p [SufB_c - a_p*C_c] rides in 2
    extra columns per chunk (C split 256q + r so the bf16 coefficients are
    exact); each term is provably >= 0 for real rows and < 0 for +BIG pads,
    so the same relu reduction handles it;
  - per batch ONE relu+row-sum instruction (ACT activation Relu accum_out or
    DVE tensor_scalar max0/add accum_out) over [128, 592] consumes the tile.
    Window splits are adaptive per batch (encoded in tile CONTENT, built on
    host); only the 592-col budget is fixed. P(batch exceeds budget) < 1e-4;
    such batches fall back to exact host compute.
  - batches are paired into [128, 1184] psum tiles (3 banks, 4 bank-aligned
    matmuls: 512|80|432|160) so PE runs long uninterrupted bursts.
  - BCE via ACT Softplus (same table set as Relu -> one ACT_TABLE_LOAD) +
    3 DVE ops; weights pre-scaled by 1/(B*N) on host.
Host: sorting/searchsorted/hi-lo encoding (layout prep), fallbacks, final
scalar in f64. Device does all O(N*band) reduction work + BCE.
"""
import sys
import numpy as np

sys.path.insert(0, "/opt/trn_rl_repo")

B, N = 64, 1024
N_CORES = 8
BLOC = B // N_CORES          # batches per core
CH = 5                       # pos-rank chunks of 128 (Pa <= 640 w.p. ~1-1e-15)
NXC = 2 * CH                 # X (remainder) columns per batch
CB = 592                     # per-batch tile budget (cols): NXC + windows
PAIRW = 2 * CB               # pair tile width (3 psum banks)
ROWS = 12                    # b hi/lo + 5x a hi/lo
SELW = BLOC * 128            # 1024 selector cols
B2W = BLOC * CB              # 4736 value cols
BFW = SELW + B2W             # combined bf16 tile width
MARGIN = 0.5
BIG = 16.0                   # pad sentinel; |b| <= ~7 << BIG

_CACHE = {}


def _patch_bass(bass):
    """Split multi-wait instructions (old walrus TPB_CTRL takes 1 wait)."""
    import json as _json
    if getattr(bass.Bass, "_wait_split_patched", False):
        return
    _orig = bass.Bass.to_json_bytes

    def _split(bir, limit=1):
        m = _json.loads(bir)
        for fn in m["functions"]:
            for bb in fn["blocks"]:
                out = []
                for i in bb.get("instructions", []):
                    si = i.get("sync_info") or {}
                    ow = si.get("on_wait") or []
                    if len(ow) > limit:
                        extra, keep = ow[:-limit], ow[-limit:]
                        for k, w in enumerate(extra):
                            out.append({
                                "debug": i.get("debug"), "engine": i["engine"],
                                "ins": [], "outs": [],
                                "name": i["name"] + f"_ws{k}",
                                "opcode": "NoOp",
                                "sync_info": {"on_wait": [w]},
                            })
                        si = dict(si)
                        si["on_wait"] = keep
                        i = dict(i)
                        i["sync_info"] = si
                    out.append(i)
                bb["instructions"] = out
        return _json.dumps(m).encode()

    bass.Bass.to_json_bytes = lambda self: _split(_orig(self))
    bass.Bass._wait_split_patched = True


def _build(bass, tile, mybir):
    f32 = mybir.dt.float32
    bf16 = mybir.dt.bfloat16
    Alu = mybir.AluOpType
    Act = mybir.ActivationFunctionType

    nc = bass.Bass()
    W1 = SELW + 4 * CB                  # sel + b2 for batches 0-3
    W2 = 4 * CB                         # b2 for batches 4-7
    bf1_d = nc.declare_dram_parameter("bf1", [ROWS, W1], bf16, isOutput=False)
    bf2_d = nc.declare_dram_parameter("bf2", [ROWS, W2], bf16, isOutput=False)
    fv_d = nc.declare_dram_parameter("fv", [128, 192], f32, isOutput=False)
    outd_d = nc.declare_dram_parameter("outd", [128, 8], f32, isOutput=True)
    outa_d = nc.declare_dram_parameter("outa", [128, 8], f32, isOutput=True)

    with tile.TileContext(nc) as tc:
        with (
            tc.tile_pool(name="const", bufs=1) as const,
            tc.tile_pool(name="work", bufs=2) as work,
            tc.tile_pool(name="psum", bufs=2, space="PSUM") as psum,
        ):
            # engine-issued DMAs: the two HWDGE configs run in parallel on
            # the DVE/ACT sequencers instead of serially on Sync
            # bf1 (sel + first 4 batches, gates PE's first pairs) configured
            # first on the ACT sequencer; bf2/fv trail and stream in behind
            bf1 = const.tile([ROWS, W1], bf16)
            nc.scalar.dma_start(out=bf1[:], in_=bf1_d[:])
            bf2 = const.tile([ROWS, W2], bf16)
            nc.scalar.dma_start(out=bf2[:], in_=bf2_d[:])
            fv = const.tile([128, 192], f32)
            nc.sync.dma_start(out=fv[:], in_=fv_d[:])

            acc_d = const.tile([128, 8], f32)
            acc_a = const.tile([128, 8], f32)
            nc.vector.memset(acc_d[:], 0.0)
            nc.vector.memset(acc_a[:], 0.0)

            # pair cols [0:1184) split 704 (DVE) / 480 (ACT) across two psum
            # tiles so the two consumers never read the SAME tile (the tile
            # framework serializes same-tile readers across engines).
            DW = 736

            def emit_mm(t):
                """Pair tiles for batches 2t, 2t+1; 4 bank-contained matmuls."""
                bcd = psum.tile([128, DW], f32, tag="bcd")
                bca = psum.tile([128, PAIRW - DW], f32, tag="bca")
                e, o = 2 * t, 2 * t + 1
                le = bf1[:, 128 * e:128 * e + 128]
                lo_ = bf1[:, 128 * o:128 * o + 128]
                src = bf1 if t < 2 else bf2
                obe = (SELW if t < 2 else 0) + CB * (e % 4)
                obo = (SELW if t < 2 else 0) + CB * (o % 4)
                nc.tensor.matmul(bcd[:, 0:512], le, src[:, obe:obe + 512],
                                 start=True, stop=True)
                nc.tensor.matmul(bcd[:, 512:592], le, src[:, obe + 512:obe + 592],
                                 start=True, stop=True)
                nc.tensor.matmul(bcd[:, 592:DW], lo_, src[:, obo:obo + (DW - 592)],
                                 start=True, stop=True)
                nc.tensor.matmul(bca[:, 0:PAIRW - DW], lo_,
                                 src[:, obo + (DW - 592):obo + CB],
                                 start=True, stop=True)
                return bcd, bca

            def ew_dve(bcd, col):
                scr = work.tile([128, DW], f32, tag="scr_d")
                nc.vector.tensor_scalar(
                    out=scr[:], in0=bcd[:], scalar1=0.0, scalar2=None,
                    op0=Alu.max, op1=Alu.add, accum_out=acc_d[:, col:col + 1])

            def ew_act(bca, col):
                scr = work.tile([128, PAIRW - DW], f32, tag="scr_a")
                nc.scalar.activation(
                    out=scr[:], in_=bca[:], func=Act.Relu,
                    accum_out=acc_a[:, col:col + 1])

            p0 = emit_mm(0)
            p1 = emit_mm(1)

            # BCE: sum w'*(softplus(v) - v*y) = sum w'*sp - sum wy*v with
            # wy = w'*y host-precomputed; softplus = Ln(Exp(v) + 1) via Ln's
            # bias (natural_log_exp table also holds Relu -> one table load)
            ex = work.tile([128, 64], f32)
            nc.scalar.activation(out=ex[:], in_=fv[:, 0:64], func=Act.Exp)
            sp = work.tile([128, 64], f32)
            nc.scalar.activation(out=sp[:], in_=ex[:], func=Act.Ln, bias=1.0)

            ew_dve(p0[0], 0)
            ew_act(p0[1], 0)

            b1 = work.tile([128, 64], f32)
            nc.vector.scalar_tensor_tensor(
                out=b1[:], in0=sp[:], scalar=1.0, op0=Alu.mult,
                op1=Alu.mult, in1=fv[:, 128:192], accum_out=acc_d[:, 4:5])
            b2s = work.tile([128, 64], f32)
            nc.vector.scalar_tensor_tensor(
                out=b2s[:], in0=fv[:, 0:64], scalar=-1.0, op0=Alu.mult,
                op1=Alu.mult, in1=fv[:, 64:128], accum_out=acc_d[:, 5:6])

            p2 = emit_mm(2)
            ew_dve(p1[0], 1)
            ew_act(p1[1], 1)
            p3 = emit_mm(3)
            ew_dve(p2[0], 2)
            ew_act(p2[1], 2)
            ew_dve(p3[0], 3)
            ew_act(p3[1], 3)

            nc.sync.dma_start(out=outd_d[:], in_=acc_d[:])
            nc.scalar.dma_start(out=outa_d[:], in_=acc_a[:])

    return nc


def _get_nc():
    if "nc" not in _CACHE:
        import concourse.bass as bass
        import concourse.tile as tile
        from concourse import mybir
        _patch_bass(bass)
        _CACHE["nc"] = _build(bass, tile, mybir)
    return _CACHE["nc"]


def _hi_lo(x):
    """f64 array -> (bf16 hi, bf16 lo) with hi+lo ~ x to ~2^-17 rel."""
    import ml_dtypes
    hi = x.astype(np.float32).astype(ml_dtypes.bfloat16)
    lo = (x - hi.astype(np.float64)).astype(np.float32).astype(ml_dtypes.bfloat16)
    return hi, lo


def _exact_mean(pos, neg):
    """Exact per-batch pairwise mean (f64); pos/neg sorted, neg has +M."""
    if len(pos) == 0 or len(neg) == 0:
        return 0.0
    dsum = 0.0
    # chunked to keep memory small
    for i0 in range(0, len(pos), 128):
        d = neg[None, :] - pos[i0:i0 + 128, None]
        dsum += float(np.maximum(d, 0.0).sum())
    return dsum / (len(pos) * len(neg))


def _prep_batch(vrow, yrow, selblk, b2blk):
    """Fill one batch's selector [ROWS,128] and value [ROWS,CB] blocks
    (f64, hi/lo split done by caller is NOT used -- we fill final f32 content
    here and caller casts). Returns (valid, fallback_mean_or_None)."""
    pos = np.sort(vrow[yrow == 1.0]).astype(np.float64)
    neg = np.sort(vrow[yrow == 0.0]).astype(np.float64) + MARGIN
    Pa, Nb = len(pos), len(neg)
    n_pairs = Pa * Nb
    if n_pairs == 0:
        return False, None            # invalid batch: zero content, mean 0
    if Pa > CH * 128:
        return True, _exact_mean(pos, neg)
    s = 1.0 / n_pairs

    W0s, needs = [], []
    for c in range(CH):
        lo_r = 128 * c
        if lo_r >= Pa:
            W0s.append(Nb)
            needs.append(0)
            continue
        hi_r = min(lo_r + 127, Pa - 1)
        w0 = int(np.searchsorted(neg, pos[lo_r], 'left'))
        hi = int(np.searchsorted(neg, pos[hi_r], 'right'))
        W0s.append(w0)
        needs.append(hi - w0)
    if sum(needs) + NXC > CB:
        return True, _exact_mean(pos, neg)

    negs = neg * s
    bval = np.zeros(CB, dtype=np.float64)     # b-row content (pre hi/lo)
    aind = np.zeros((CH, CB), dtype=np.float64)  # a-row indicator/coef rows
    avals = np.full((CH, 128), -s * BIG, dtype=np.float64)  # -s*a per chunk

    col = NXC
    for c in range(CH):
        w0, nd = W0s[c], needs[c]
        lo_r = 128 * c
        cnt = max(0, min(128, Pa - lo_r))
        if cnt > 0:
            avals[c, :cnt] = -s * pos[lo_r:lo_r + cnt]
        if nd > 0:
            bval[col:col + nd] = negs[w0:w0 + nd]
            aind[c, col:col + nd] = 1.0
        # remainder: C fully-active neg above the window
        E = w0 + nd
        C = Nb - E
        if C > 0:
            sufb = float(negs[E:].sum())
            q, r = C >> 8, C & 255
            xq, xr = 2 * c, 2 * c + 1
            if q > 0:
                bval[xq] = sufb * (256.0 * q / C)
                aind[c, xq] = 256.0 * q
            if r > 0:
                bval[xr] = sufb * (r / C)
                aind[c, xr] = r
        col += nd

    bhi, blo = _hi_lo(bval)
    b2blk[0, :] = bhi
    b2blk[1, :] = blo
    ahi, alo = _hi_lo(avals)
    for c in range(CH):
        b2blk[2 + 2 * c, :] = aind[c].astype(np.float32)
        b2blk[3 + 2 * c, :] = aind[c].astype(np.float32)
        selblk[2 + 2 * c, :] = ahi[c]
        selblk[3 + 2 * c, :] = alo[c]
    selblk[0, :] = 1.0
    selblk[1, :] = 1.0
    return True, None


def make_in_maps(v, y, w):
    import ml_dtypes
    in_maps, aux = [], []
    wsc = (w.astype(np.float64) / (B * N)).astype(np.float32)
    for core in range(N_CORES):
        sl = slice(core * BLOC, (core + 1) * BLOC)
        vb, yb, wb = v[sl], y[sl], wsc[sl]
        bft = np.zeros((ROWS, BFW), dtype=ml_dtypes.bfloat16)
        extra_mean = 0.0
        n_valid = 0
        for b in range(BLOC):
            selblk = np.zeros((ROWS, 128), dtype=ml_dtypes.bfloat16)
            b2blk = np.zeros((ROWS, CB), dtype=ml_dtypes.bfloat16)
            valid, fb = _prep_batch(vb[b], yb[b], selblk, b2blk)
            if valid:
                n_valid += 1
            if fb is not None:
                extra_mean += fb      # fallback: host-exact, zero content
            else:
                bft[:, 128 * b:128 * b + 128] = selblk
                bft[:, SELW + CB * b:SELW + CB * (b + 1)] = b2blk
        wy = (wb.astype(np.float64) * yb).astype(np.float32)
        fvt = np.concatenate(
            [vb.reshape(128, 64), wy.reshape(128, 64), wb.reshape(128, 64)],
            axis=1).astype(np.float32)
        w1 = SELW + 4 * CB
        in_maps.append({"bf1": np.ascontiguousarray(bft[:, :w1]),
                        "bf2": np.ascontiguousarray(bft[:, w1:]),
                        "fv": np.ascontiguousarray(fvt)})
        aux.append({"extra_mean": extra_mean, "n_valid": n_valid})
    return in_maps, aux


def kernel(logits, labels, weights):
    from concourse.bass_utils import run_bass_kernel_spmd

    nc = _get_nc()
    v = np.ascontiguousarray(logits.reshape(B, N), dtype=np.float32)
    y = np.ascontiguousarray(labels.reshape(B, N), dtype=np.float32)
    w = np.ascontiguousarray(weights.reshape(B, N), dtype=np.float32)

    in_maps, aux = make_in_maps(v, y, w)
    res = run_bass_kernel_spmd(nc, in_maps, list(range(N_CORES)))

    mean_sum = 0.0
    bce_sum = 0.0
    valid_count = 0
    for c in range(N_CORES):
        od = np.asarray(res.results[c]["outd"]).astype(np.float64)
        oa = np.asarray(res.results[c]["outa"]).astype(np.float64)
        mean_sum += od[:, 0:4].sum() + oa[:, 0:4].sum()
        bce_sum += od[:, 4].sum() + od[:, 5].sum()
        mean_sum += aux[c]["extra_mean"]
        valid_count += aux[c]["n_valid"]
    rank_loss = mean_sum / valid_count if valid_count > 0 else 0.0
    return np.float32(bce_sum + rank_loss)

